# revision 1
# baseline (speedup 1.0000x reference)
"""MelSpectrogram + PCEN Trainium2 kernel (8-core data parallel).

Pipeline per core (8 batch elements):
  host: reflect-pad, hop-block transpose (512 x 2528), fp16 cast
  PE:   hop-block DFT via matmul (E [512 x 2304 f-slots], fp16, 1/16-scaled)
  DVE:  butterfly combination A/X (4-block overlap-add with (-i)^f phases)
  DVE/GPSIMD: frequency-domain hann (3-tap conv via scalar_tensor_tensor), square
  PE:   mel projection (fb folded with comp-duplication + scale)
  PE:   PCEN IIR smoother as lower-triangular Toeplitz matmul (via PE transpose)
  ACT:  PCEN pointwise via ln/exp chain
All elementwise data fp16; matmul accumulation fp32.

f-slot layout per component (r=cos, i=-sin), 9 tiles of 128 slots each:
  p0: f=4q     -> slot q       (q=0..256; slots 257..383 zero-pad)
  p1: f=4q+1   -> slot 384+q   (q=0..255)
  p2: f=4q+2   -> slot 640+q
  p3: f=4q+3   -> slot 896+q
Component r occupies tiles 0..8, component i tiles 9..17.
"""

import math
from contextlib import ExitStack

import numpy as np

SR, N_FFT, HOP, N_MELS = 32000, 2048, 512, 128
F_MIN, F_MAX = 20.0, 16000.0
EPS, S, ALPHA, DELTA, R = 1e-6, 0.025, 0.98, 2.0, 0.5
NBINS = N_FFT // 2 + 1
T = 313          # frames per batch element
SBLK = 316       # hop blocks per batch element
PAD = N_FFT // 2
B_TOTAL, L_WAVE = 64, 160000
N_CORES = 8

SC = 16.0    # E scale (E = E_true/SC)
SCM = 16.0   # mel scale (mel' = mel_true/SCM)
SCL = 8.0    # L scale (LT = L*SCL)
SCE = 256.0  # e2 scale (keeps (eps+m)^-alpha in fp16 normal range)
NSLOT = 1152
W_CHUNK = 384


_BASES = {("r", 0): 0, ("r", 2): 384, ("i", 0): 640, ("i", 2): 1024,
          ("r", 1): 1280, ("r", 3): 1536, ("i", 1): 1792, ("i", 3): 2048}


def _slot_of(f, comp):
    p, q = f % 4, f // 4
    return _BASES[(comp, p)] + q


def _mel_fbank():
    def hz2mel(f):
        return 2595.0 * np.log10(1.0 + np.asarray(f, np.float64) / 700.0)

    def mel2hz(m):
        return 700.0 * (10.0 ** (np.asarray(m, np.float64) / 2595.0) - 1.0)

    all_freqs = np.linspace(0.0, SR / 2.0, NBINS)
    m_pts = np.linspace(hz2mel(F_MIN), hz2mel(F_MAX), N_MELS + 2)
    f_pts = mel2hz(m_pts)
    f_diff = np.diff(f_pts)
    slopes = f_pts[None, :] - all_freqs[:, None]
    down = -slopes[:, :-2] / f_diff[:-1]
    up = slopes[:, 2:] / f_diff[1:]
    return np.maximum(0.0, np.minimum(down, up))  # [1025, 128]


def _build_consts():
    r = np.arange(512)
    E = np.zeros((512, 2 * NSLOT), np.float64)
    for f in range(NBINS):
        th = 2.0 * np.pi * f * r / N_FFT
        E[:, _slot_of(f, "r")] = np.cos(th) / SC
        E[:, _slot_of(f, "i")] = -np.sin(th) / SC
    fb = _mel_fbank()
    fb2 = np.zeros((2 * NSLOT, N_MELS), np.float64)
    for f in range(NBINS):
        w = fb[f] * (SC * SC / 4.0) / SCM
        fb2[_slot_of(f, "r")] = w
        fb2[_slot_of(f, "i")] = w
    LT = np.zeros((384, T), np.float64)
    t = np.arange(T)
    for tau in range(T):
        m = t >= tau
        LT[tau, m] = S * (1.0 - S) ** (t[m] - tau) * SCL
    return E, fb2, LT


def _split_multiwaits(nc, limit=1):
    """This walrus build accepts at most `limit` sync-waits per instruction;
    move excess waits onto preceding same-engine NoOps."""
    import bass_rust
    import concourse.mybir as mybir

    for fn in nc.m.functions:
        for b in fn.blocks:
            insts = b.instructions
            new = []
            changed = False
            for i in insts:
                si = i.sync_info
                if si is not None and len(si.on_wait) > limit:
                    waits = list(si.on_wait)
                    for k in range(0, len(waits) - limit, limit):
                        chunk = waits[k : k + limit]
                        nop = mybir.InstNoOp(
                            name=f"{i.name}-wsplit{k}", ins=[], outs=[]
                        )
                        nop.engine = i.engine
                        nop.sync_info = bass_rust.SyncInfo(
                            on_wait=chunk, on_update=[]
                        )
                        new.append(nop)
                        changed = True
                    si.on_wait = waits[len(waits) - limit :]
                new.append(i)
            if changed:
                b.instructions = new


def build_nc(BL=8, W=W_CHUNK, split=True):
    """Build the Bass program for one core processing BL batch elements."""
    import concourse.bass as bass
    import concourse.mybir as mybir
    from concourse import tile

    f16 = mybir.dt.float16
    f32 = mybir.dt.float32
    ALU = mybir.AluOpType
    ACTF = mybir.ActivationFunctionType

    NC = BL * SBLK
    stride = W - 3
    starts = list(range(0, NC - 3, stride))
    # clip last chunk
    chunks = []
    for co in starts:
        w = min(W, NC - co)
        if w < 8:
            continue
        chunks.append((co, w))

    nc = bass.Bass("TRN2", target_bir_lowering=False, debug=False)
    xt_d = nc.dram_tensor("xt", [4, 128, NC], f16, kind="ExternalInput")
    e_d = nc.dram_tensor("e", [4, 128, 2304], f16, kind="ExternalInput")
    fb_d = nc.dram_tensor("fb", [128, 2304], f16, kind="ExternalInput")
    lt_d = nc.dram_tensor("lt", [3, 128, T], f16, kind="ExternalInput")
    id_d = nc.dram_tensor("idn", [128, 128], f16, kind="ExternalInput")
    sh_d = nc.dram_tensor("sh", [128, 512], f16, kind="ExternalInput")
    y_d = nc.dram_tensor("y", [BL, 128, T], f32, kind="ExternalOutput")

    with tile.TileContext(nc) as tc, ExitStack() as top:
        cpool = top.enter_context(tc.tile_pool(name="consts", bufs=1))
        xb = cpool.tile([128, 4 * NC], f16)
        eb = cpool.tile([128, 4 * 2304], f16)
        fbb = cpool.tile([128, 2304], f16)
        ltb = cpool.tile([128, 3 * T], f16)
        idb = cpool.tile([128, 128], f16)
        shb = cpool.tile([128, 512], f16)
        melb = cpool.tile([128, NC], f16)

        xbv = xb[:, :].rearrange("p (rc c) -> p rc c", rc=4)
        ebv = eb[:, :].rearrange("p (rc c) -> p rc c", rc=4)
        ltv = ltb[:, :].rearrange("p (k t) -> p k t", k=3)
        for rc in range(4):
            nc.sync.dma_start(xbv[:, rc, :], xt_d.ap()[rc])
            nc.sync.dma_start(ebv[:, rc, :], e_d.ap()[rc])
        nc.sync.dma_start(fbb[:, :], fb_d.ap()[:, :])
        for k in range(3):
            nc.sync.dma_start(ltv[:, k, :], lt_d.ap()[k])
        nc.sync.dma_start(idb[:, :], id_d.ap()[:, :])
        nc.sync.dma_start(shb[:, :], sh_d.ap()[:, :])

        # ---------------- chunk phase ----------------
        with ExitStack() as cph:
            yps = cph.enter_context(tc.tile_pool(name="yps", bufs=3, space="PSUM"))
            mps = cph.enter_context(tc.tile_pool(name="mps", bufs=2, space="PSUM"))
            p_ysb = cph.enter_context(tc.tile_pool(name="p_ysb", bufs=2))
            p_a = cph.enter_context(tc.tile_pool(name="p_a", bufs=2))
            p_x = cph.enter_context(tc.tile_pool(name="p_x", bufs=2))
            p_xw = cph.enter_context(tc.tile_pool(name="p_xw", bufs=1))
            p_h = cph.enter_context(tc.tile_pool(name="p_h", bufs=1))
            p_pw = cph.enter_context(tc.tile_pool(name="p_pw", bufs=1))
            xsps = cph.enter_context(tc.tile_pool(name="xsps", bufs=3, space="PSUM"))

            WT = 18 * W

            def emit_dft(co, w):
                ysb = p_ysb.tile([128, WT], f16, tag="ysb")
                if w < W:
                    # short last chunk: zero tile tails the flat ops read
                    nc.gpsimd.memset(ysb[:, :], 0.0)
                for ft in range(18):
                    yp = yps.tile([128, W], f32, tag="yp")
                    for rc in range(4):
                        nc.tensor.matmul(
                            yp[:, :w],
                            eb[:, :].rearrange("p (rc c) -> p rc c", rc=4)[
                                :, rc, ft * 128 : (ft + 1) * 128
                            ],
                            xb[:, :].rearrange("p (rc c) -> p rc c", rc=4)[
                                :, rc, co : co + w
                            ],
                            start=(rc == 0),
                            stop=(rc == 3),
                        )
                    if ft % 4 == 0:
                        nc.vector.tensor_copy(ysb[:, ft * W : ft * W + w], yp[:, :w])
                    else:
                        nc.scalar.copy(ysb[:, ft * W : ft * W + w], yp[:, :w])
                return ysb

            ysb_next = emit_dft(*chunks[0])
            for ci, (co, w) in enumerate(chunks):
                V = w - 3
                ysb = ysb_next
                if ci + 1 < len(chunks):
                    # emit next chunk's DFT first so the PE stream never
                    # waits on this chunk's elementwise stages
                    ysb_next = emit_dft(*chunks[ci + 1])
                # A-step: A_t = Y_t +/- Y_{t+2} (sign-grouped layout: tiles
                # 0..9 are p0/p2 -> '+', tiles 10..17 are p1/p3 -> '-').
                # Flat 1D ops; cross-tile contamination lands in the 3
                # garbage tail cols of each tile, never read downstream.
                a = p_a.tile([128, WT], f16)
                nc.vector.tensor_add(
                    a[:, 0 : 10 * W], ysb[:, 0 : 10 * W], ysb[:, 2 : 10 * W + 2]
                )
                nc.vector.tensor_sub(
                    a[:, 10 * W : WT - 2], ysb[:, 10 * W : WT - 2], ysb[:, 10 * W + 2 : WT]
                )
                nc.gpsimd.memset(a[:, WT - 2 : WT], 0.0)

                # X-step: X_t = A_t + (-i)^f A_{t+1}
                # plane tile ranges: (comp, p) -> (tile0, ntiles)
                PL = {("r", 0): (0, 3), ("r", 2): (3, 2), ("i", 0): (5, 3),
                      ("i", 2): (8, 2), ("r", 1): (10, 2), ("r", 3): (12, 2),
                      ("i", 1): (14, 2), ("i", 3): (16, 2)}
                x = p_x.tile([128, WT], f16)
                xsteps = [
                    (("r", 0), ("r", 0), True), (("i", 0), ("i", 0), True),
                    (("r", 2), ("r", 2), False), (("i", 2), ("i", 2), False),
                    (("r", 1), ("i", 1), True), (("i", 1), ("r", 1), False),
                    (("r", 3), ("i", 3), False), (("i", 3), ("r", 3), True),
                ]
                engs = [nc.vector, nc.vector, nc.vector, nc.gpsimd,
                        nc.gpsimd, nc.gpsimd, nc.gpsimd, nc.gpsimd]
                for (outp, srcp, pos), eng in zip(xsteps, engs):
                    o0, nt = PL[outp]
                    s0, _ = PL[srcp]
                    op = eng.tensor_add if pos else eng.tensor_sub
                    ext = 0 if s0 + nt >= 18 else 1
                    n = nt * W - 1 + ext
                    op(
                        x[:, o0 * W : o0 * W + n],
                        a[:, o0 * W : o0 * W + n],
                        a[:, s0 * W + 1 : s0 * W + 1 + n],
                    )
                    if not ext:
                        nc.gpsimd.memset(
                            x[:, o0 * W + n : o0 * W + nt * W], 0.0
                        )

                # cross-plane q+-1 shifted copies of p3 (down) / p0 (up)
                # via PE shift-matrix matmuls into PSUM.
                shv = shb[:, :].rearrange("p (k c) -> p k c", k=4)
                SDN, CFIRST, SUP, CLAST = 0, 1, 2, 3
                xs3 = {}
                xs0 = {}
                for comp in ("r", "i"):
                    p3t, _ = PL[(comp, 3)]
                    p0t, _ = PL[(comp, 0)]
                    t3a = xsps.tile([128, W], f32, tag="xs")
                    nc.tensor.matmul(t3a[:, :], shv[:, SDN, :],
                                     x[:, p3t * W : p3t * W + W],
                                     start=True, stop=True)
                    t3b = xsps.tile([128, W], f32, tag="xs")
                    nc.tensor.matmul(t3b[:, :], shv[:, SDN, :],
                                     x[:, (p3t + 1) * W : (p3t + 2) * W],
                                     start=True, stop=False)
                    nc.tensor.matmul(t3b[:, :], shv[:, CFIRST, :],
                                     x[:, p3t * W : p3t * W + W],
                                     start=False, stop=True)
                    t0a = xsps.tile([128, W], f32, tag="xs")
                    nc.tensor.matmul(t0a[:, :], shv[:, SUP, :],
                                     x[:, p0t * W : p0t * W + W],
                                     start=True, stop=False)
                    nc.tensor.matmul(t0a[:, :], shv[:, CLAST, :],
                                     x[:, (p0t + 1) * W : (p0t + 2) * W],
                                     start=False, stop=True)
                    t0b = xsps.tile([128, W], f32, tag="xs")
                    nc.tensor.matmul(t0b[:, :], shv[:, SUP, :],
                                     x[:, (p0t + 1) * W : (p0t + 2) * W],
                                     start=True, stop=False)
                    nc.tensor.matmul(t0b[:, :], shv[:, CLAST, :],
                                     x[:, (p0t + 2) * W : (p0t + 3) * W],
                                     start=False, stop=True)
                    xs3[comp] = (t3a, t3b)
                    xs0[comp] = (t0a, t0b)

                # wconv: tmp = X - H[f-1]; xw = tmp - H[f+1], H = 0.5*X
                # (H via single-src tensor_scalar at 4x; shift matrices carry
                # the 0.5 for the cross-plane PSUM pieces)
                h = p_h.tile([128, WT], f16)
                nc.vector.tensor_scalar_mul(h[:, :], x[:, :], 0.5)
                tmp = p_a.tile([128, WT], f16, tag="a")
                xw = p_xw.tile([128, WT], f16)

                def rng(t0, nt):
                    return slice(t0 * W, t0 * W + nt * W)

                sub = nc.vector.tensor_sub
                for comp in ("r", "i"):
                    p0, _ = PL[(comp, 0)]
                    p1, _ = PL[(comp, 1)]
                    p2, _ = PL[(comp, 2)]
                    p3, _ = PL[(comp, 3)]
                    sub(tmp[:, rng(p1, 2)], x[:, rng(p1, 2)], h[:, rng(p0, 2)])
                    sub(tmp[:, rng(p2, 2)], x[:, rng(p2, 2)], h[:, rng(p1, 2)])
                    sub(tmp[:, rng(p3, 2)], x[:, rng(p3, 2)], h[:, rng(p2, 2)])
                    for k in range(2):
                        sub(tmp[:, rng(p0 + k, 1)], x[:, rng(p0 + k, 1)],
                            xs3[comp][k][:, :])
                    sub(xw[:, rng(p0, 2)], tmp[:, rng(p0, 2)], h[:, rng(p1, 2)])
                    sub(xw[:, rng(p1, 2)], tmp[:, rng(p1, 2)], h[:, rng(p2, 2)])
                    sub(xw[:, rng(p2, 2)], tmp[:, rng(p2, 2)], h[:, rng(p3, 2)])
                    for k in range(2):
                        sub(xw[:, rng(p3 + k, 1)], tmp[:, rng(p3 + k, 1)],
                            xs0[comp][k][:, :])
                    nc.gpsimd.memset(xw[:, rng(p0 + 2, 1)], 0.0)

                # pw = xw^2 (split DVE / ACT)
                pw = p_pw.tile([128, WT], f16)
                half = 6 * W
                nc.vector.tensor_mul(pw[:, 0:half], xw[:, 0:half], xw[:, 0:half])
                nc.scalar.activation(pw[:, half:WT], xw[:, half:WT], ACTF.Square)

                # mel projection
                mp = mps.tile([128, W], f32, tag="mp")
                mel_fts = [ft for ft in range(18) if ft not in (2, 7)]
                for j, ft in enumerate(mel_fts):
                    nc.tensor.matmul(
                        mp[:, 0:V],
                        fbb[:, ft * 128 : (ft + 1) * 128],
                        pw[:, ft * W : ft * W + V],
                        start=(j == 0),
                        stop=(j == len(mel_fts) - 1),
                    )
                nc.scalar.copy(melb[:, co : co + V], mp[:, 0:V])

        # ---------------- tail phase: PCEN ----------------
        with ExitStack() as tph:
            tps = tph.enter_context(tc.tile_pool(name="tps", bufs=2, space="PSUM"))
            msp_pool = tph.enter_context(
                tc.tile_pool(name="msp", bufs=2, space="PSUM")
            )
            ppool = tph.enter_context(tc.tile_pool(name="pcen", bufs=1))
            melT = ppool.tile([128, BL * 3 * 128], f16)
            e1b = ppool.tile([128, BL * T], f32)
            e2b = ppool.tile([128, BL * T], f32)
            e3b = ppool.tile([128, BL * T], f16)
            e4b = ppool.tile([128, BL * T], f32)
            e5b = ppool.tile([128, BL * T], f32)
            outb = ppool.tile([128, BL * T], f32)
            bias_t = ppool.tile([128, 3], f32)
            nc.vector.memset(bias_t[:, 0:1], EPS)
            nc.vector.memset(bias_t[:, 1:2], math.log(SCE))
            nc.vector.memset(bias_t[:, 2:3], DELTA)

            tlens = (128, 128, 57)
            for b in range(BL):
                for k in range(3):
                    tl = tlens[k]
                    tp = tps.tile([128, 128], f16, tag="tp")
                    nc.tensor.transpose(
                        tp[0:tl, :],
                        melb[:, b * SBLK + k * 128 : b * SBLK + k * 128 + tl],
                        idb[:, :],
                    )
                    nc.vector.tensor_copy(
                        melT[0:tl, (b * 3 + k) * 128 : (b * 3 + k + 1) * 128],
                        tp[0:tl, :],
                    )
                msp = msp_pool.tile([128, T], f32, tag="ms")
                for k in range(3):
                    tl = tlens[k]
                    nc.tensor.matmul(
                        msp[:, :],
                        melT[0:tl, (b * 3 + k) * 128 : (b * 3 + k + 1) * 128],
                        ltv[0:tl, k, :],
                        start=(k == 0),
                        stop=(k == 2),
                    )
                # e1 = ln(ms*SCM/SCL + EPS)
                nc.scalar.activation(
                    e1b[:, b * T : (b + 1) * T],
                    msp[:, :],
                    ACTF.Ln,
                    bias=bias_t[:, 0:1],
                    scale=SCM / SCL,
                )
            # e2 = exp(-alpha*e1 + ln(SCE))  (= SCE*(eps+m)^-alpha)
            nc.scalar.activation(
                e2b[:, :], e1b[:, :], ACTF.Exp, bias=bias_t[:, 1:2], scale=-ALPHA
            )
            # e3 = e2 * mel' (valid frames only)
            melv = melb[:, :].rearrange("p (b c) -> p b c", b=BL)
            e2v = e2b[:, :].rearrange("p (b c) -> p b c", b=BL)
            e3v = e3b[:, :].rearrange("p (b c) -> p b c", b=BL)
            nc.vector.tensor_mul(e3v[:, :, :], e2v[:, :, :], melv[:, :, 0:T])
            # e4 = ln(e3*SCM/SCE + DELTA)
            nc.scalar.activation(
                e4b[:, :], e3b[:, :], ACTF.Ln, bias=bias_t[:, 2:3], scale=SCM / SCE
            )
            # e5 = exp(R*e4); out = e5 - DELTA^R
            nc.scalar.activation(e5b[:, :], e4b[:, :], ACTF.Exp, scale=R)
            nc.vector.tensor_scalar_add(outb[:, :], e5b[:, :], -(DELTA**R))
            outv = outb[:, :].rearrange("p (b c) -> p b c", b=BL)
            yv = y_d.ap().rearrange("b m t -> m b t")
            nc.sync.dma_start(yv[:, :, :], outv[:, :, :])

    if split:
        _split_multiwaits(nc)
    return nc


# ---------------------------------------------------------------- host side

_CACHE = {}


def _get_consts():
    if "consts" not in _CACHE:
        E, fb2, LT = _build_consts()
        e_h = np.ascontiguousarray(
            E.astype(np.float16).reshape(4, 128, 2304), dtype=np.float16
        )
        # fb tile layout: fb_h[p, j*128+m] = fb2[j*128+p, m]
        fb_h = np.ascontiguousarray(
            fb2.astype(np.float16).reshape(18, 128, 128).transpose(1, 0, 2)
            .reshape(128, 2304)
        )
        lt_h = np.ascontiguousarray(
            LT.astype(np.float16).reshape(3, 128, T), dtype=np.float16
        )
        id_h = np.eye(128, dtype=np.float16)
        sdn = 0.5 * np.eye(128, k=1)
        cfirst = np.zeros((128, 128)); cfirst[127, 0] = 0.5
        sup = 0.5 * np.eye(128, k=-1)
        clast = np.zeros((128, 128)); clast[0, 127] = 0.5
        sh_h = np.ascontiguousarray(
            np.concatenate([sdn, cfirst, sup, clast], axis=1).astype(np.float16)
        )
        _CACHE["consts"] = (e_h, fb_h, lt_h, id_h, sh_h)
    return _CACHE["consts"]


def _prep_core_input(wf_core):
    """wf_core: [BL, 160000] f32 -> xt [4, 128, BL*316] f16."""
    BL = wf_core.shape[0]
    x = np.pad(wf_core, ((0, 0), (PAD, PAD)), mode="reflect")
    blocks = x[:, : SBLK * HOP].reshape(BL, SBLK, HOP)          # [BL, 316, 512]
    xT = blocks.transpose(2, 0, 1).reshape(HOP, BL * SBLK)      # [512, BL*316]
    return np.ascontiguousarray(
        xT.astype(np.float16).reshape(4, 128, BL * SBLK)
    )


def _get_nc():
    if "nc" not in _CACHE:
        _CACHE["nc"] = build_nc(BL=8)
    return _CACHE["nc"]


def kernel(waveform: np.ndarray) -> np.ndarray:
    from concourse.bass_utils import run_bass_kernel_spmd

    waveform = np.asarray(waveform, np.float32)
    assert waveform.shape == (B_TOTAL, L_WAVE)
    e_h, fb_h, lt_h, id_h, sh_h = _get_consts()
    BL = B_TOTAL // N_CORES
    in_maps = []
    for c in range(N_CORES):
        xt = _prep_core_input(waveform[c * BL : (c + 1) * BL])
        in_maps.append(
            {"xt": xt, "e": e_h, "fb": fb_h, "lt": lt_h, "idn": id_h,
             "sh": sh_h}
        )
    nc = _get_nc()
    res = run_bass_kernel_spmd(nc, in_maps, core_ids=list(range(N_CORES)))
    out = np.empty((B_TOTAL, 1, N_MELS, T), np.float32)
    for c in range(N_CORES):
        y = np.asarray(res.results[c]["y"])  # [BL, 128, T]
        out[c * BL : (c + 1) * BL, 0] = y
    return out

